# revision 11
# baseline (speedup 1.0000x reference)
"""Multi-head attention (B=4, S=2048, E=1024, H=16) on 8 TRN2 NeuronCores.

Sharding: tensor-parallel over heads. Core c computes output columns
[128c, 128c+128) (heads 2c and 2c+1). Inputs q,v are fed to every core
pre-transposed to [B, E, S] so projection matmuls can use the token dim
as the moving (N=512) operand; W*/b* are column-sliced per core.

On-chip layout (per core, per batch):
  qpT/kpT/vpT [128 (= 2 heads x 64 d), 2048 tok]  -- projections, transposed
  scoresT [k_tok, q_tok] so exp+AV contraction keeps k on partitions
  softmax sums come from a ones-column appended to token-major v (M=66
  augmented AV matmul; col 65 is zero padding for the fp32r even-width
  rule); no max subtraction (scores ~ N(0, 0.25^2)).

All matmuls run in float32r (TF32-like, full PE rate at N>=512). fp32r
operands must be produced as fp32r, so matmul-feeding tiles are fp32r
and host inputs are pre-rounded.
"""

import numpy as np
from contextlib import ExitStack

import concourse.bass as bass
import concourse.tile as tile
from concourse import bacc, mybir
from concourse.bass_utils import run_bass_kernel_spmd

B, SQ, SK, E, H = 4, 2048, 2048, 1024, 16
NCORES = 8
CPC = E // NCORES          # output cols per core = 128
D = E // H                 # head dim = 64
NE = E // 128              # contraction tiles for projections = 8
NKT = SK // 128            # k-token tiles = 16
NQB = SQ // 512            # q blocks of 512 = 4
SCALE = 1.0 / np.sqrt(E)   # faithful to reference: 1/sqrt(embed_dim)

F32 = mybir.dt.float32
F32R = mybir.dt.float32r
EXP = mybir.ActivationFunctionType.Exp


def _body(ctx: ExitStack, tc: "tile.TileContext", out, qT, vT, wq, wk, wv,
          bq, bk, bv, ident, aug_const):
    nc = tc.nc

    const = ctx.enter_context(tc.tile_pool(name="const", bufs=1))
    stream = ctx.enter_context(tc.tile_pool(name="stream", bufs=10))
    qp_pool = ctx.enter_context(tc.tile_pool(name="qp", bufs=2))
    kp_pool = ctx.enter_context(tc.tile_pool(name="kp", bufs=2))
    qp1_pool = ctx.enter_context(tc.tile_pool(name="qp1", bufs=2))
    kp1_pool = ctx.enter_context(tc.tile_pool(name="kp1", bufs=2))
    vp_pool = ctx.enter_context(tc.tile_pool(name="vp", bufs=2))
    aug_pool = ctx.enter_context(tc.tile_pool(name="aug", bufs=32))
    exp_pool = ctx.enter_context(tc.tile_pool(name="exp", bufs=4))
    avs_pool = ctx.enter_context(tc.tile_pool(name="avs", bufs=4))
    rec_pool = ctx.enter_context(tc.tile_pool(name="rec", bufs=4))
    out_pool = ctx.enter_context(tc.tile_pool(name="outp", bufs=8))
    psum_sc = ctx.enter_context(tc.tile_pool(name="psc", bufs=2, space="PSUM"))
    psum_av = ctx.enter_context(tc.tile_pool(name="pav", bufs=2, space="PSUM"))
    psum_pp = ctx.enter_context(tc.tile_pool(name="ppp", bufs=2, space="PSUM"))

    # --- constants: weight tiles [128 (E-slice), 128 (out col)], biases, identity
    w_sb = {}
    for pname, wdram in (("q", wq), ("k", wk), ("v", wv)):
        for e in range(NE):
            t = const.tile([128, CPC], F32R, tag=f"w{pname}{e}")
            nc.sync.dma_start(t[:], wdram[e * 128:(e + 1) * 128, :])
            w_sb[(pname, e)] = t
    b_sb = {}
    for pname, bdram in (("q", bq), ("k", bk), ("v", bv)):
        t = const.tile([CPC, 1], F32, tag=f"b{pname}")
        nc.sync.dma_start(t[:], bdram[:, :])
        b_sb[pname] = t
    id_sb = const.tile([128, 128], F32, tag="ident")
    nc.sync.dma_start(id_sb[:], ident[:, :])
    # [1, 0] per partition, fp32r (memset cannot produce fp32r)
    ones2_sb = const.tile([128, 2], F32R, tag="ones2")
    nc.sync.dma_start(ones2_sb[:], aug_const[:, :])

    for b in range(B):
        # ---------------- projections ----------------
        qpT = qp_pool.tile([CPC, SQ], F32R)
        kpT = kp_pool.tile([CPC, SK], F32R)
        vpT = vp_pool.tile([CPC, SK], F32)
        for jp in range(2):  # 1024-token block pairs (4KB DMA descriptors)
            jps = slice(jp * 1024, (jp + 1) * 1024)
            # v feeds both k- and v-projections
            vtiles = []
            for e in range(NE):
                t = stream.tile([128, 1024], F32R, tag="in", name=f"vin{e}")
                nc.sync.dma_start(t[:], vT[b, e * 128:(e + 1) * 128, jps])
                vtiles.append(t)
            for pname, dst in (("k", kpT), ("v", vpT)):
                for j2 in range(2):
                    js = slice(jp * 1024 + j2 * 512, jp * 1024 + j2 * 512 + 512)
                    rs = slice(j2 * 512, (j2 + 1) * 512)
                    pp = psum_pp.tile([128, 512], F32, tag="pp")
                    for e in range(NE):
                        nc.tensor.matmul(pp[:], w_sb[(pname, e)][:],
                                         vtiles[e][:, rs],
                                         start=(e == 0), stop=(e == NE - 1))
                    nc.vector.tensor_scalar_add(dst[:, js], pp[:],
                                                b_sb[pname][:])
            qtiles = []
            for e in range(NE):
                t = stream.tile([128, 1024], F32R, tag="in", name=f"qin{e}")
                nc.sync.dma_start(t[:], qT[b, e * 128:(e + 1) * 128, jps])
                qtiles.append(t)
            for j2 in range(2):
                js = slice(jp * 1024 + j2 * 512, jp * 1024 + j2 * 512 + 512)
                rs = slice(j2 * 512, (j2 + 1) * 512)
                pp = psum_pp.tile([128, 512], F32, tag="pp")
                for e in range(NE):
                    nc.tensor.matmul(pp[:], w_sb[("q", e)][:],
                                     qtiles[e][:, rs],
                                     start=(e == 0), stop=(e == NE - 1))
                nc.vector.tensor_scalar_add(qpT[:, js], pp[:], b_sb["q"][:])
        # head-1 halves at base partition 0 (base-64 matmul operands are slow)
        kpT1 = kp1_pool.tile([64, SK], F32R)
        qpT1 = qp1_pool.tile([64, SQ], F32R)
        nc.sync.dma_start(kpT1[:], kpT[64:128, :])
        nc.sync.dma_start(qpT1[:], qpT[64:128, :])

        # ---------------- v -> token-major with ones column ----------------
        # vh_aug[t]: [128 k-tok, 132]:
        #   [h0 d(0:64) | 1s | 0pad | h1 d(66:130) | 1s | 0pad]
        vh_aug = []
        for t in range(NKT):
            tp = psum_pp.tile([128, 128], F32, tag="pp")
            nc.tensor.transpose(tp[:], vpT[:, t * 128:(t + 1) * 128], id_sb[:])
            a = aug_pool.tile([128, 132], F32R, tag="aug")
            nc.vector.tensor_copy(a[:, 0:64], tp[:, 0:64])
            nc.vector.tensor_copy(a[:, 66:130], tp[:, 64:128])
            nc.vector.tensor_copy(a[:, 64:66], ones2_sb[:])
            nc.vector.tensor_copy(a[:, 130:132], ones2_sb[:])
            vh_aug.append(a)

        # ---------------- attention ----------------
        for qb in range(NQB):
            qs = slice(qb * 512, (qb + 1) * 512)
            av0 = psum_av.tile([66, 512], F32, tag="av")
            av1 = psum_av.tile([66, 512], F32, tag="av")
            for kt in range(NKT):
                ks = slice(kt * 128, (kt + 1) * 128)
                ps = psum_sc.tile([128, 1024], F32)
                # per-head K=64 matmuls, both operands at base partition 0
                nc.tensor.matmul(ps[:, 0:512], kpT[0:64, ks],
                                 qpT[0:64, qs], start=True, stop=True)
                nc.tensor.matmul(ps[:, 512:1024], kpT1[:, ks],
                                 qpT1[:, qs], start=True, stop=True)
                et = exp_pool.tile([128, 1024], F32R)
                nc.scalar.activation(et[:], ps[:], EXP, scale=SCALE)
                nc.tensor.matmul(av0[:], vh_aug[kt][:, 0:66],
                                 et[:, 0:512],
                                 start=(kt == 0), stop=(kt == NKT - 1))
                nc.tensor.matmul(av1[:], vh_aug[kt][:, 66:132],
                                 et[:, 512:1024],
                                 start=(kt == 0), stop=(kt == NKT - 1))
            # normalize + emit [128 q, 128 cols] tiles
            ots = [out_pool.tile([128, 128], F32, tag="ot", name=f"ot{t}")
                   for t in range(4)]
            for h, av in ((0, av0), (1, av1)):
                avs = avs_pool.tile([66, 512], F32, tag="avs")
                nc.vector.tensor_copy(avs[:], av[:])
                for t in range(4):
                    tp = psum_pp.tile([128, 66], F32, tag="pp")
                    nc.tensor.transpose(tp[:], avs[:, t * 128:(t + 1) * 128],
                                        id_sb[0:66, 0:66])
                    # reciprocal on [128,1] is cheap (DVE divide is 8 cyc/elem)
                    rec = rec_pool.tile([128, 1], F32, tag="rec")
                    nc.vector.reciprocal(rec[:], tp[:, 64:65])
                    nc.vector.tensor_scalar_mul(
                        ots[t][:, h * 64:(h + 1) * 64], tp[:, 0:64],
                        rec[:])
            for t in range(4):
                r0 = qb * 512 + t * 128
                nc.sync.dma_start(out[b, r0:r0 + 128, :], ots[t][:])


_CACHE = {}


def _build():
    if "nc" in _CACHE:
        return _CACHE["nc"]
    nc = bacc.Bacc("TRN2", target_bir_lowering=False, debug=False,
                   enable_asserts=False)
    qT = nc.dram_tensor("qT", [B, E, SQ], F32R, kind="ExternalInput").ap()
    vT = nc.dram_tensor("vT", [B, E, SK], F32R, kind="ExternalInput").ap()
    wq = nc.dram_tensor("wq", [E, CPC], F32R, kind="ExternalInput").ap()
    wk = nc.dram_tensor("wk", [E, CPC], F32R, kind="ExternalInput").ap()
    wv = nc.dram_tensor("wv", [E, CPC], F32R, kind="ExternalInput").ap()
    bq = nc.dram_tensor("bq", [CPC, 1], F32, kind="ExternalInput").ap()
    bk = nc.dram_tensor("bk", [CPC, 1], F32, kind="ExternalInput").ap()
    bv = nc.dram_tensor("bv", [CPC, 1], F32, kind="ExternalInput").ap()
    ident = nc.dram_tensor("ident", [128, 128], F32, kind="ExternalInput").ap()
    aug_const = nc.dram_tensor("aug_const", [128, 2], F32R,
                               kind="ExternalInput").ap()
    out = nc.dram_tensor("out", [B, SQ, CPC], F32, kind="ExternalOutput").ap()
    with tile.TileContext(nc) as tc:
        with ExitStack() as ctx:
            _body(ctx, tc, out, qT, vT, wq, wk, wv, bq, bk, bv, ident, aug_const)
    nc.compile()
    _CACHE["nc"] = nc
    return nc


def _round_tf32(x):
    """Round-to-nearest-even to 10 explicit mantissa bits (TF32)."""
    u = np.ascontiguousarray(x, np.float32).view(np.uint32)
    r = (u + 0x1000 + ((u >> 13) & 1)) & np.uint32(0xFFFFE000)
    return r.view(np.float32)


def _in_maps(q, v, Wq, bq, Wk, bk, Wv, bv):
    f = np.float32
    qT = _round_tf32(np.transpose(np.asarray(q, f), (0, 2, 1)))
    vT = _round_tf32(np.transpose(np.asarray(v, f), (0, 2, 1)))
    ident = np.eye(128, dtype=f)
    aug2 = np.tile(np.array([1.0, 0.0], f), (128, 1))
    Wq, Wk, Wv = (np.asarray(x, f) for x in (Wq, Wk, Wv))
    bq, bk, bv = (np.asarray(x, f) for x in (bq, bk, bv))
    maps = []
    for c in range(NCORES):
        sl = slice(c * CPC, (c + 1) * CPC)
        maps.append({
            "qT": qT, "vT": vT, "ident": ident, "aug_const": aug2,
            "wq": _round_tf32(Wq[:, sl]),
            "wk": _round_tf32(Wk[:, sl]),
            "wv": _round_tf32(Wv[:, sl]),
            "bq": np.ascontiguousarray(bq[sl]).reshape(CPC, 1),
            "bk": np.ascontiguousarray(bk[sl]).reshape(CPC, 1),
            "bv": np.ascontiguousarray(bv[sl]).reshape(CPC, 1),
        })
    return maps


def run(trace=False, **inputs):
    nc = _build()
    maps = _in_maps(**inputs)
    res = run_bass_kernel_spmd(nc, maps, core_ids=list(range(NCORES)),
                               trace=trace)
    full = np.concatenate([res.results[c]["out"] for c in range(NCORES)],
                          axis=2)
    return full, res


def kernel(q, v, Wq, bq, Wk, bk, Wv, bv):
    full, _ = run(q=q, v=v, Wq=Wq, bq=bq, Wk=Wk, bk=bk, Wv=Wv, bv=bv)
    return full


# revision 15
# speedup vs baseline: 1.4216x; 1.4216x over previous
"""Multi-head attention (B=4, S=2048, E=1024, H=16) on 8 TRN2 NeuronCores.

Sharding: tensor-parallel over heads. Core c computes output columns
[128c, 128c+128) (heads 2c and 2c+1). Inputs q,v are fed to every core
pre-transposed to [B, E, S] so projection matmuls can use the token dim
as the moving (N=512) operand; W*/b* are column-sliced per core.

On-chip layout (per core, per batch):
  qpT/kpT/vpT [128 (= 2 heads x 64 d), 2048 tok]  -- projections, transposed
  scoresT [k_tok, q_tok] so exp+AV contraction keeps k on partitions
  softmax sums come from a ones-column appended to token-major v (M=66
  augmented AV matmul; col 65 is zero padding for the fp32r even-width
  rule); no max subtraction (scores ~ N(0, 0.25^2)).

All matmuls run in float32r (TF32-like, full PE rate at N>=512). fp32r
operands must be produced as fp32r, so matmul-feeding tiles are fp32r
and host inputs are pre-rounded.
"""

import numpy as np
from contextlib import ExitStack

import concourse.bass as bass
import concourse.tile as tile
from concourse import bacc, mybir
from concourse.bass_utils import run_bass_kernel_spmd

B, SQ, SK, E, H = 4, 2048, 2048, 1024, 16
NCORES = 8
CPC = E // NCORES          # output cols per core = 128
D = E // H                 # head dim = 64
NE = E // 128              # contraction tiles for projections = 8
NKT = SK // 128            # k-token tiles = 16
NQB = SQ // 512            # q blocks of 512 = 4
SCALE = 1.0 / np.sqrt(E)   # faithful to reference: 1/sqrt(embed_dim)

F32 = mybir.dt.float32
F32R = mybir.dt.float32r
EXP = mybir.ActivationFunctionType.Exp


def _body(ctx: ExitStack, tc: "tile.TileContext", out, qT, vT, wq, wk, wv,
          bq, bk, bv, ident, aug_const):
    nc = tc.nc

    const = ctx.enter_context(tc.tile_pool(name="const", bufs=1))
    stream = ctx.enter_context(tc.tile_pool(name="stream", bufs=11))
    qp_pool = ctx.enter_context(tc.tile_pool(name="qp", bufs=2))
    kp_pool = ctx.enter_context(tc.tile_pool(name="kp", bufs=2))
    qp1_pool = ctx.enter_context(tc.tile_pool(name="qp1", bufs=2))
    kp1_pool = ctx.enter_context(tc.tile_pool(name="kp1", bufs=2))
    vp_pool = ctx.enter_context(tc.tile_pool(name="vp", bufs=2))
    aug_pool = ctx.enter_context(tc.tile_pool(name="aug", bufs=32))
    exp_pool = ctx.enter_context(tc.tile_pool(name="exp", bufs=4))
    avs_pool = ctx.enter_context(tc.tile_pool(name="avs", bufs=4))
    rec_pool = ctx.enter_context(tc.tile_pool(name="rec", bufs=4))
    out_pool = ctx.enter_context(tc.tile_pool(name="outp", bufs=8))
    psum_sc = ctx.enter_context(tc.tile_pool(name="psc", bufs=2, space="PSUM"))
    psum_av = ctx.enter_context(tc.tile_pool(name="pav", bufs=2, space="PSUM"))
    psum_pp = ctx.enter_context(tc.tile_pool(name="ppp", bufs=2, space="PSUM"))

    # --- constants: weight tiles [128 (E-slice), 128 (out col)], biases, identity
    w_sb = {}
    for pname, wdram in (("q", wq), ("k", wk), ("v", wv)):
        for e in range(NE):
            t = const.tile([128, CPC], F32R, tag=f"w{pname}{e}")
            nc.sync.dma_start(t[:], wdram[e * 128:(e + 1) * 128, :])
            w_sb[(pname, e)] = t
    b_sb = {}
    for pname, bdram in (("q", bq), ("k", bk), ("v", bv)):
        t = const.tile([CPC, 1], F32, tag=f"b{pname}")
        nc.sync.dma_start(t[:], bdram[:, :])
        b_sb[pname] = t
    id_sb = const.tile([128, 128], F32, tag="ident")
    nc.sync.dma_start(id_sb[:], ident[:, :])
    # [1, 0] per partition, fp32r (memset cannot produce fp32r)
    ones2_sb = const.tile([128, 2], F32R, tag="ones2")
    nc.sync.dma_start(ones2_sb[:], aug_const[:, :])

    for b in range(B):
        # ---------------- projections ----------------
        qpT = qp_pool.tile([CPC, SQ], F32R)
        kpT = kp_pool.tile([CPC, SK], F32R)
        vpT = vp_pool.tile([CPC, SK], F32)
        # head-1 halves at base partition 0 (base-64 matmul operands are slow)
        kpT1 = kp1_pool.tile([64, SK], F32R)
        qpT1 = qp1_pool.tile([64, SQ], F32R)
        for jp in range(2):  # 1024-token block pairs (4KB DMA descriptors)
            jps = slice(jp * 1024, (jp + 1) * 1024)
            # v feeds both k- and v-projections
            vtiles = []
            for e in range(NE):
                t = stream.tile([128, 1024], F32R, tag="in", name=f"vin{e}")
                nc.sync.dma_start(t[:], vT[b, e * 128:(e + 1) * 128, jps])
                vtiles.append(t)
            for pname, dst in (("k", kpT), ("v", vpT)):
                for j2 in range(2):
                    js = slice(jp * 1024 + j2 * 512, jp * 1024 + j2 * 512 + 512)
                    rs = slice(j2 * 512, (j2 + 1) * 512)
                    pp = psum_pp.tile([128, 512], F32, tag="pp")
                    for e in range(NE):
                        nc.tensor.matmul(pp[:], w_sb[(pname, e)][:],
                                         vtiles[e][:, rs],
                                         start=(e == 0), stop=(e == NE - 1))
                    nc.vector.tensor_scalar_add(dst[:, js], pp[:],
                                                b_sb[pname][:])
                    if pname == "k":
                        nc.sync.dma_start(kpT1[:, js], kpT[64:128, js])
            qtiles = []
            for e in range(NE):
                t = stream.tile([128, 1024], F32R, tag="in", name=f"qin{e}")
                nc.sync.dma_start(t[:], qT[b, e * 128:(e + 1) * 128, jps])
                qtiles.append(t)
            for j2 in range(2):
                js = slice(jp * 1024 + j2 * 512, jp * 1024 + j2 * 512 + 512)
                rs = slice(j2 * 512, (j2 + 1) * 512)
                pp = psum_pp.tile([128, 512], F32, tag="pp")
                for e in range(NE):
                    nc.tensor.matmul(pp[:], w_sb[("q", e)][:],
                                     qtiles[e][:, rs],
                                     start=(e == 0), stop=(e == NE - 1))
                nc.vector.tensor_scalar_add(qpT[:, js], pp[:], b_sb["q"][:])
                nc.sync.dma_start(qpT1[:, js], qpT[64:128, js])

        # ---------------- v -> token-major with ones column ----------------
        # vh_aug[t]: [128 k-tok, 132]:
        #   [h0 d(0:64) | 1s | 0pad | h1 d(66:130) | 1s | 0pad]
        vh_aug = []
        for t in range(NKT):
            tp = psum_pp.tile([128, 128], F32, tag="pp")
            nc.tensor.transpose(tp[:], vpT[:, t * 128:(t + 1) * 128], id_sb[:])
            a = aug_pool.tile([128, 132], F32R, tag="aug")
            nc.vector.tensor_copy(a[:, 0:64], tp[:, 0:64])
            nc.vector.tensor_copy(a[:, 66:130], tp[:, 64:128])
            nc.vector.tensor_copy(a[:, 64:66], ones2_sb[:])
            nc.vector.tensor_copy(a[:, 130:132], ones2_sb[:])
            vh_aug.append(a)

        # ---------------- attention ----------------
        for qb in range(NQB):
            qs = slice(qb * 512, (qb + 1) * 512)
            av0 = psum_av.tile([66, 512], F32, tag="av")
            av1 = psum_av.tile([66, 512], F32, tag="av")
            for kt in range(NKT):
                ks = slice(kt * 128, (kt + 1) * 128)
                ps = psum_sc.tile([128, 1024], F32)
                # per-head K=64 matmuls, both operands at base partition 0
                nc.tensor.matmul(ps[:, 0:512], kpT[0:64, ks],
                                 qpT[0:64, qs], start=True, stop=True)
                nc.tensor.matmul(ps[:, 512:1024], kpT1[:, ks],
                                 qpT1[:, qs], start=True, stop=True)
                et = exp_pool.tile([128, 1024], F32R)
                nc.scalar.activation(et[:], ps[:], EXP, scale=SCALE)
                nc.tensor.matmul(av0[:], vh_aug[kt][:, 0:66],
                                 et[:, 0:512],
                                 start=(kt == 0), stop=(kt == NKT - 1))
                nc.tensor.matmul(av1[:], vh_aug[kt][:, 66:132],
                                 et[:, 512:1024],
                                 start=(kt == 0), stop=(kt == NKT - 1))
            # normalize + emit [128 q, 128 cols] tiles
            ots = [out_pool.tile([128, 128], F32, tag="ot", name=f"ot{t}")
                   for t in range(4)]
            for h, av in ((0, av0), (1, av1)):
                avs = avs_pool.tile([66, 512], F32, tag="avs")
                nc.vector.tensor_copy(avs[:], av[:])
                for t in range(4):
                    tp = psum_pp.tile([128, 66], F32, tag="pp")
                    nc.tensor.transpose(tp[:], avs[:, t * 128:(t + 1) * 128],
                                        id_sb[0:66, 0:66])
                    # reciprocal on [128,1] is cheap (DVE divide is 8 cyc/elem)
                    rec = rec_pool.tile([128, 1], F32, tag="rec")
                    nc.vector.reciprocal(rec[:], tp[:, 64:65])
                    nc.vector.tensor_scalar_mul(
                        ots[t][:, h * 64:(h + 1) * 64], tp[:, 0:64],
                        rec[:])
            for t in range(4):
                r0 = qb * 512 + t * 128
                nc.sync.dma_start(out[b, r0:r0 + 128, :], ots[t][:])


_CACHE = {}


def _build():
    if "nc" in _CACHE:
        return _CACHE["nc"]
    nc = bacc.Bacc("TRN2", target_bir_lowering=False, debug=False,
                   enable_asserts=False)
    qT = nc.dram_tensor("qT", [B, E, SQ], F32R, kind="ExternalInput").ap()
    vT = nc.dram_tensor("vT", [B, E, SK], F32R, kind="ExternalInput").ap()
    wq = nc.dram_tensor("wq", [E, CPC], F32R, kind="ExternalInput").ap()
    wk = nc.dram_tensor("wk", [E, CPC], F32R, kind="ExternalInput").ap()
    wv = nc.dram_tensor("wv", [E, CPC], F32R, kind="ExternalInput").ap()
    bq = nc.dram_tensor("bq", [CPC, 1], F32, kind="ExternalInput").ap()
    bk = nc.dram_tensor("bk", [CPC, 1], F32, kind="ExternalInput").ap()
    bv = nc.dram_tensor("bv", [CPC, 1], F32, kind="ExternalInput").ap()
    ident = nc.dram_tensor("ident", [128, 128], F32, kind="ExternalInput").ap()
    aug_const = nc.dram_tensor("aug_const", [128, 2], F32R,
                               kind="ExternalInput").ap()
    out = nc.dram_tensor("out", [B, SQ, CPC], F32, kind="ExternalOutput").ap()
    with tile.TileContext(nc) as tc:
        with ExitStack() as ctx:
            _body(ctx, tc, out, qT, vT, wq, wk, wv, bq, bk, bv, ident, aug_const)
    nc.compile()
    _CACHE["nc"] = nc
    return nc


def _round_tf32(x):
    """Round-to-nearest-even to 10 explicit mantissa bits (TF32)."""
    u = np.ascontiguousarray(x, np.float32).view(np.uint32)
    r = (u + 0x1000 + ((u >> 13) & 1)) & np.uint32(0xFFFFE000)
    return r.view(np.float32)


def _in_maps(q, v, Wq, bq, Wk, bk, Wv, bv):
    f = np.float32
    qT = _round_tf32(np.transpose(np.asarray(q, f), (0, 2, 1)))
    vT = _round_tf32(np.transpose(np.asarray(v, f), (0, 2, 1)))
    ident = np.eye(128, dtype=f)
    aug2 = np.tile(np.array([1.0, 0.0], f), (128, 1))
    Wq, Wk, Wv = (np.asarray(x, f) for x in (Wq, Wk, Wv))
    bq, bk, bv = (np.asarray(x, f) for x in (bq, bk, bv))
    maps = []
    for c in range(NCORES):
        sl = slice(c * CPC, (c + 1) * CPC)
        maps.append({
            "qT": qT, "vT": vT, "ident": ident, "aug_const": aug2,
            "wq": _round_tf32(Wq[:, sl]),
            "wk": _round_tf32(Wk[:, sl]),
            "wv": _round_tf32(Wv[:, sl]),
            "bq": np.ascontiguousarray(bq[sl]).reshape(CPC, 1),
            "bk": np.ascontiguousarray(bk[sl]).reshape(CPC, 1),
            "bv": np.ascontiguousarray(bv[sl]).reshape(CPC, 1),
        })
    return maps


def run(trace=False, **inputs):
    nc = _build()
    maps = _in_maps(**inputs)
    res = run_bass_kernel_spmd(nc, maps, core_ids=list(range(NCORES)),
                               trace=trace)
    full = np.concatenate([res.results[c]["out"] for c in range(NCORES)],
                          axis=2)
    return full, res


def kernel(q, v, Wq, bq, Wk, bk, Wv, bv):
    full, _ = run(q=q, v=v, Wq=Wq, bq=bq, Wk=Wk, bk=bk, Wv=Wv, bv=bv)
    return full
